# revision 49
# baseline (speedup 1.0000x reference)
"""Trainium2 Bass kernel for the PraxisGraph MoE-routing module.

Strategy (8 NeuronCores, data-parallel over the sequence axis):
  - hidden_states [4, 4096, 2048] is sharded along S: each core streams its
    [4, 512, 2048] shard, computes per-token LayerNorm statistics and
    accumulates the per-batch sum of normalized tokens via TensorE matmuls
    (a-weighted token sum, with the mu*a correction folded in afterwards).
  - The routing head is algebraically folded: since k = (L+C) @ Wk.T + bk
    depends only on (tiny) parameters, attention = q @ k.T / sqrt(H) is an
    affine map of the token-sum: att = S1_tot @ W_att + b_att, with
    W_att = diag(gamma) Wq.T k.T / (S sqrt(H)) precomputed host-side in f64.
    Spatial/edge biases and the avail mask fold into a per-expert bias.
  - By linearity each core computes its local partial attention [4, 16]
    before the collective; one 8-core AllReduce of 64 B combines them.
  - Each core then computes softmax and the KL routing loss on device;
    core 0's outputs are returned. argmax is done on host.
"""

import numpy as np
from contextlib import ExitStack

import concourse.bacc as bacc
import concourse.tile as tile
from concourse import mybir
from concourse.bass_utils import run_bass_kernel_spmd
from concourse.masks import make_identity

F32 = mybir.dt.float32
F16 = mybir.dt.float16
F32R = mybir.dt.float32r
AF = mybir.ActivationFunctionType
ALU = mybir.AluOpType

NCORES = 8
B, S, H, E = 4, 4096, 2048, 16
P = 128
SL = S // NCORES           # 512 sequence positions per core
C = H // P                 # 16 hidden chunks of 128
TILES = B * SL // P        # 16 [128, H] token tiles per core
LN_EPS = 1e-5
ROUTING_SCALE = 0.01
NEG_INF = -1e9

_NC = None          # cached compiled program
_LAST_RESULT = None  # BassKernelResults of the last run (for profiling)


def _build(spmd=True, cut=None):
    nc = bacc.Bacc("TRN2", target_bir_lowering=False, debug=False,
                   num_devices=NCORES if spmd else 1)
    x = nc.dram_tensor("x", [B, SL, H], F16, kind="ExternalInput").ap()
    watt = nc.dram_tensor("watt", [P, C * E], F32, kind="ExternalInput").ap()
    bscore = nc.dram_tensor("bscore", [B, E], F32, kind="ExternalInput").ap()
    consts = nc.dram_tensor("consts", [1, 2], F32, kind="ExternalInput").ap()
    out_o = nc.dram_tensor("out", [B, E + 1], F32, kind="ExternalOutput").ap()

    with tile.TileContext(nc) as tc, ExitStack() as ctx:
        xin = ctx.enter_context(tc.tile_pool(name="xin", bufs=6))
        small = ctx.enter_context(tc.tile_pool(name="small", bufs=4))
        persist = ctx.enter_context(tc.tile_pool(name="persist", bufs=1))
        ps_big = ctx.enter_context(tc.tile_pool(name="ps_big", bufs=1, space="PSUM"))
        ps_small = ctx.enter_context(tc.tile_pool(name="ps_small", bufs=1, space="PSUM"))
        dram = ctx.enter_context(tc.tile_pool(name="dram", bufs=1, space="DRAM"))

        # --- constants / persistent state ---
        eps_t = persist.tile([P, 1], F32)
        nc.vector.memset(eps_t, LN_EPS)
        ones_t = persist.tile([P, 1], F32)
        nc.vector.memset(ones_t, 1.0)
        ident = persist.tile([P, P], F32)
        make_identity(nc, ident)
        watt_sb = persist.tile([P, C * E], F32)
        nc.sync.dma_start(watt_sb, watt)
        bscore_sb = persist.tile([B, E], F32)
        nc.sync.dma_start(bscore_sb, bscore)
        cvals = persist.tile([1, 2], F32)
        nc.sync.dma_start(cvals, consts)
        csum_acc = persist.tile([P, B], F32)
        nc.vector.memset(csum_acc, 0.0)
        ones_row = persist.tile([1, B], F32)
        nc.vector.memset(ones_row, 1.0)
        # one-hot column masks: mask_t[:, b, :] has 1.0 in column b
        mask_t = persist.tile([P, B, B], F32)
        nc.vector.memset(mask_t, 0.0)
        for b in range(B):
            nc.vector.memset(mask_t[:, b, b:b + 1], 1.0)

        S1 = ps_big.tile([B, H], F32)  # 4 PSUM banks: per-batch weighted token sums

        # --- phase 1: stream the hidden shard, accumulate LN token sums ---
        for tix in range(TILES):
            b = tix // (TILES // B)
            i = tix % (TILES // B)
            # hidden is streamed as fp16: quantization noise averages out
            # over 4096 tokens x 2048 dims (measured probs delta 2.8e-7) and
            # halves the HBM traffic; fp16 matmuls run at 1 cyc/row with no
            # fp32r rounding requirement, so the PE consumes xt directly.
            xt = xin.tile([P, H], F16)
            for q in range(4):
                nc.sync.dma_start(xt[:, q * (H // 4):(q + 1) * (H // 4)],
                                  x[b, i * P:(i + 1) * P,
                                    q * (H // 4):(q + 1) * (H // 4)])

            stats = small.tile([P, H // 512, nc.vector.BN_STATS_DIM], F32)
            for j in range(H // 512):
                nc.vector.bn_stats(stats[:, j, :], xt[:, j * 512:(j + 1) * 512])
            mv = small.tile([P, nc.vector.BN_AGGR_DIM], F32)
            nc.vector.bn_aggr(mv, stats)

            # std = sqrt(var + eps); Sqrt shares its ACT function table with
            # Copy/Identity, so phase 1 needs a single table load.
            std = small.tile([P, 1], F32)
            nc.scalar.activation(std, mv[:, 1:2], AF.Sqrt, bias=eps_t)
            a = small.tile([P, 1], F32)
            nc.vector.reciprocal(a, std)
            # c = a * mu
            c_t = small.tile([P, 1], F32)
            nc.scalar.activation(c_t, a, AF.Copy, scale=mv[:, 0:1])

            lhsT = small.tile([P, B], F16)
            nc.scalar.activation(lhsT, mask_t[:, b, :], AF.Copy, scale=a)
            nc.vector.tensor_add(csum_acc[:, b:b + 1], csum_acc[:, b:b + 1], c_t)

            for n in range(H // 512):
                nc.tensor.matmul(S1[:, n * 512:(n + 1) * 512], lhsT,
                                 xt[:, n * 512:(n + 1) * 512],
                                 start=(tix == 0), stop=(tix == TILES - 1))
        # --- phase 1b: correction term, local partial attention ---
        csum_ps = ps_small.tile([B, 1], F32)
        nc.tensor.matmul(csum_ps, csum_acc, ones_t, start=True, stop=True)
        ncsum = small.tile([B, 1], F32)
        nc.vector.tensor_scalar_mul(ncsum, csum_ps, -1.0)
        s_sb = persist.tile([B, H], F32)
        nc.scalar.activation(s_sb, S1, AF.Identity, bias=ncsum)
        if cut == "phase1":
            nc.sync.dma_start(out_o, s_sb[:, :E + 1])
            nc.compile()
            return nc

        # att_part = S_local @ W_att  (4x16); AllReduce over cores is linear
        mT_ps = ps_small.tile([P, C, B], F32)
        for c in range(C):
            nc.tensor.transpose(mT_ps[:, c, :], s_sb[:, c * P:(c + 1) * P],
                                ident[:B, :B])
        mT = persist.tile([P, C, B], F32)
        nc.vector.tensor_copy(mT, mT_ps)

        att_ps = ps_small.tile([B, E], F32)
        for c in range(C):
            nc.tensor.matmul(att_ps, mT[:, c, :], watt_sb[:, c * E:(c + 1) * E],
                             start=(c == 0), stop=(c == C - 1))
        att_sb = small.tile([B, E], F32)
        nc.scalar.activation(att_sb, att_ps, AF.Copy)
        if cut == "att":
            nc.sync.dma_start(out_o[:, :E], att_sb)
            nc.compile()
            return nc
        cc_in = dram.tile([B, E], F32)
        cc_out = dram.tile([B, E], F32, addr_space="Shared")
        nc.sync.dma_start(cc_in, att_sb)
        if spmd:
            nc.gpsimd.collective_compute(
                "AllReduce", ALU.add, replica_groups=[list(range(NCORES))],
                ins=[cc_in.opt()], outs=[cc_out.opt()],
            )
        else:
            nc.sync.dma_start(cc_out, cc_in)
        att_full = small.tile([B, E], F32)
        nc.sync.dma_start(att_full, cc_out)
        if cut == "cc":
            cpt = small.tile([B, E], F32)
            nc.vector.tensor_copy(cpt, att_full)
            nc.sync.dma_start(out_o[:, :E], cpt)
            nc.compile()
            return nc

        # --- phase 2: scores, softmax, loss ---
        scores = small.tile([B, E], F32)
        nc.vector.tensor_add(scores, att_full, bscore_sb)
        nmax = small.tile([B, 1], F32)
        nc.vector.reduce_max(nmax, scores, axis=mybir.AxisListType.X,
                             negate=True)
        e_sb = small.tile([B, E], F32)
        nc.scalar.activation(e_sb, scores, AF.Exp, bias=nmax)
        z = small.tile([B, 1], F32)
        nc.vector.reduce_sum(z, e_sb, axis=mybir.AxisListType.X)
        rz = small.tile([B, 1], F32)
        nc.vector.reciprocal(rz, z)
        # combined output tile: cols 0..15 = probs, col 16 row 0 = loss
        comb = small.tile([B, E + 1], F32)
        nc.vector.memset(comb[:, E:E + 1], 0.0)
        nc.vector.tensor_scalar_mul(comb[:, :E], e_sb, rz)

        # psafe = max(e*rz, 1e-10) directly from e_sb so the loss chain does
        # not serialize behind the probs output path
        psafe = small.tile([B, E], F32)
        nc.vector.tensor_scalar(psafe, e_sb, rz, 1e-10, op0=ALU.mult, op1=ALU.max)
        logp = small.tile([B, E], F32)
        nc.scalar.activation(logp, psafe, AF.Ln)
        lsum = small.tile([B, 1], F32)
        nc.vector.reduce_sum(lsum, logp, axis=mybir.AxisListType.X)
        tot_ps = ps_small.tile([1, 1], F32)
        nc.tensor.matmul(tot_ps, lsum, ones_t[:B, :], start=True, stop=True)
        nc.scalar.activation(comb[0:1, E:E + 1], tot_ps, AF.Identity,
                             bias=cvals[:, 0:1], scale=cvals[:, 1:2])
        nc.sync.dma_start(out_o, comb)

    nc.compile()
    return nc


def _get_nc():
    global _NC
    if _NC is None:
        _NC = _build()
    return _NC


def kernel(hidden_states, layer_embeddings, centrality_embeddings,
           spatial_embeddings, edge_embeddings, ln_gamma, ln_beta,
           Wq, bq, Wk, bk, Wv, bv, current_layer, available_mask):
    global _LAST_RESULT
    hs = np.asarray(hidden_states, dtype=np.float32).astype(np.float16)
    cl = int(np.asarray(current_layer))
    avail = np.asarray(available_mask).astype(np.float64)

    gamma = np.asarray(ln_gamma, dtype=np.float64)
    beta = np.asarray(ln_beta, dtype=np.float64)
    ce = np.asarray(layer_embeddings, dtype=np.float64)[cl]
    lf = (np.asarray(layer_embeddings, dtype=np.float64)
          + np.asarray(centrality_embeddings, dtype=np.float64))
    k_full = lf @ np.asarray(Wk, dtype=np.float64).T + np.asarray(bk, np.float64)

    rsH = 1.0 / np.sqrt(H)
    wqT_kT = np.asarray(Wq, dtype=np.float64).T @ k_full.T            # [H, E]
    w_att = (gamma / S * rsH)[:, None] * wqT_kT                        # [H, E]
    b_att = (((ce + beta) @ np.asarray(Wq, np.float64).T)
             + np.asarray(bq, np.float64)) @ k_full.T * rsH            # [E]

    idx = np.arange(E)
    dist = np.abs(idx - cl)
    spatial = np.asarray(spatial_embeddings, np.float64)[dist, 0]
    ee = np.asarray(edge_embeddings, np.float64)
    ew = ee @ ee.T
    row_w = avail * (idx != cl) / np.maximum(dist, 1)
    edge_b = row_w @ ew
    mask_add = np.where(avail > 0, 0.0, NEG_INF)
    b_score = (b_att + spatial + edge_b + mask_add).astype(np.float32)

    navail = avail.sum()
    t = 1.0 / navail
    c0 = ROUTING_SCALE * E * t * np.log(t)
    c1 = -ROUTING_SCALE * t / B

    watt_dev = np.ascontiguousarray(
        w_att.astype(np.float32).reshape(C, P, E).transpose(1, 0, 2).reshape(P, C * E))
    bscore_dev = np.ascontiguousarray(np.broadcast_to(b_score, (B, E)))
    consts_dev = np.array([[c0, c1]], dtype=np.float32)

    nc = _get_nc()
    in_maps = []
    for c in range(NCORES):
        in_maps.append({
            "x": np.ascontiguousarray(hs[:, c * SL:(c + 1) * SL, :]),
            "watt": watt_dev,
            "bscore": bscore_dev,
            "consts": consts_dev,
        })
    res = run_bass_kernel_spmd(nc, in_maps, list(range(NCORES)))
    _LAST_RESULT = res

    out = np.asarray(res.results[0]["out"], dtype=np.float32)
    probs = np.ascontiguousarray(out[:, :E])
    loss = np.float32(out[0, E])
    next_idx = np.int32(np.argmax(probs[0]))
    return loss, probs, next_idx
